# revision 2
# baseline (speedup 1.0000x reference)
"""Trainium2 Bass kernel for nn_PhysicsPriorExtractor.

Reference computation per batch element (B=32768, T=64, K*D=30 features):
  spatial = pose reshaped [T, 30]                          -> out channels 0..29
  vel     = bidirectional-EMA(diff(spatial)/clip(dt))      -> out channels 30..59
  mzeni   = dot(ankle_l - ankle_r, forward_dir) + 1e-6     -> out channel 60

Strategy (pure data-parallel, batch sharded 8 ways):
  * batch on SBUF partitions, CPT=2 batch rows per partition per tile
  * EMA scans via the DVE TensorTensorScan instruction, all 30 features
    (x CPT batches) in ONE scan: feature-major [F, T] free layout with the
    recurrence reset at segment starts via data0=0 there (state = d0*state + d1)
  * backward scan runs on a time-reversed copy (ACT engine negative-stride
    copy); the combine pass reads it back reversed
  * everything assembled into a [128, CPT*64*61] output tile, one big
    contiguous DMA per tile -> memory-bound roofline
"""

import sys

if "/opt/trn_rl_repo" not in sys.path:
    sys.path.insert(0, "/opt/trn_rl_repo")

import numpy as np

B, T, F = 32768, 64, 30
OC = 2 * F + 1  # 61 output channels
N_CORES = 8
BC = B // N_CORES  # 4096 batch rows per core
CPT = 2  # batch rows per partition per tile
ROWS = 128 * CPT  # batch rows per tile
ALPHA = 0.7
Q = 1.0 - ALPHA
MAX_DT = 0.1

FT = F * T  # 1920, per-batch free width (scan layout)
W = CPT * FT  # 3840
OW = CPT * T * OC  # 7808


def build_nc(bc=BC):
    """Build the single-core Bass program processing a [bc, T, F] shard."""
    import concourse.bacc as bacc
    import concourse.mybir as mybir
    from concourse import tile
    from concourse.ap import AP

    f32 = mybir.dt.float32
    Alu = mybir.AluOpType
    n_tiles = bc // ROWS
    assert n_tiles * ROWS == bc

    nc = bacc.Bacc(
        "TRN2", target_bir_lowering=False, debug=False, num_devices=N_CORES
    )
    pose = nc.dram_tensor("pose", [bc, FT], f32, kind="ExternalInput")
    dts = nc.dram_tensor("dt", [bc, T], f32, kind="ExternalInput")
    outd = nc.dram_tensor("out", [bc, T * OC], f32, kind="ExternalOutput")

    def mk(t, off, pairs):
        # custom free-dim access pattern on a 2D [128, N] tile
        return AP(t.tensor, t.offset + off, [list(t.ap[0])] + [list(p) for p in pairs])

    with tile.TileContext(nc) as tc:
        with (
            tc.tile_pool(name="cpool", bufs=1) as cpool,
            tc.tile_pool(name="pin", bufs=3) as pin,
            tc.tile_pool(name="pmid", bufs=2) as pmid,
            tc.tile_pool(name="pout", bufs=2) as pout,
            tc.tile_pool(name="psmall", bufs=2) as psmall,
        ):
            # scan decay tile: Q everywhere, 0 at each segment start so the
            # recurrence resets per (batch-chunk, feature) segment
            qt = cpool.tile([128, W], f32)
            nc.vector.memset(qt[:, :], Q)
            nc.vector.memset(mk(qt, 0, [[FT, CPT], [T, F]]), 0.0)

            for i in range(n_tiles):
                r0 = i * ROWS
                xin = pin.tile([128, W], f32)
                dtt = psmall.tile([128, CPT * T], f32)
                rr = psmall.tile([128, CPT * T], f32)
                av = pmid.tile([128, W], f32)
                avr = pmid.tile([128, W], f32)
                outt = pout.tile([128, OW], f32)

                # ---- loads (ACT-issued HWDGE ring, separate from stores) ----
                pose_sl = pose[r0 : r0 + ROWS, :].rearrange(
                    "(c p) f -> p c f", p=128
                )
                nc.scalar.dma_start(out=mk(xin, 0, [[FT, CPT], [1, FT]]), in_=pose_sl)
                dt_sl = dts[r0 : r0 + ROWS, :].rearrange("(c p) t -> p c t", p=128)
                nc.scalar.dma_start(out=mk(dtt, 0, [[T, CPT], [1, T]]), in_=dt_sl)

                # ---- rr = 1 / clip(dt, 1e-6, MAX_DT) ----
                nc.vector.tensor_scalar(
                    out=rr[:, :], in0=dtt[:, :], scalar1=MAX_DT, scalar2=1e-6,
                    op0=Alu.min, op1=Alu.max,
                )
                nc.vector.reciprocal(rr[:, :], rr[:, :])

                # ---- av[c, f, t] = (x[c, t, f] - x[c, t-1, f]) * (0.5*alpha/dt_t)
                # x is t-major in xin; av is written f-major (T contiguous)
                av_b = mk(av, 1, [[FT, CPT], [T, F], [1, T - 1]])
                nc.vector.tensor_tensor(
                    out=av_b,
                    in0=mk(xin, F, [[FT, CPT], [1, F], [F, T - 1]]),
                    in1=mk(xin, 0, [[FT, CPT], [1, F], [F, T - 1]]),
                    op=Alu.subtract,
                )
                # TensorScalarPtr ops are limited to 3D APs by the BIR
                # verifier -> one scale op per batch chunk
                for c in range(CPT):
                    avc = mk(av, c * FT + 1, [[T, F], [1, T - 1]])
                    nc.vector.scalar_tensor_tensor(
                        out=avc, in0=avc, scalar=0.5 * ALPHA,
                        in1=mk(rr, c * T + 1, [[0, F], [1, T - 1]]),
                        op0=Alu.mult, op1=Alu.mult,
                    )
                # vel_0 = 0
                nc.vector.memset(mk(av, 0, [[FT, CPT], [T, F]]), 0.0)

                # ---- time-reversed copy for the backward scan (ACT) ----
                nc.scalar.copy(
                    out=mk(avr, 0, [[FT, CPT], [T, F], [1, T]]),
                    in_=mk(av, T - 1, [[FT, CPT], [T, F], [-1, T]]),
                )
                # backward initial condition: z_{T-1} = v_{T-1} (not alpha*v):
                # patch segment starts from 0.5*alpha*v to 0.5*v
                seg0 = mk(avr, 0, [[FT, CPT], [T, F]])
                nc.vector.tensor_scalar_mul(out=seg0, in0=seg0, scalar1=1.0 / ALPHA)

                # ---- the two EMA scans (in-place), state = q*state + d1 ----
                nc.vector.tensor_tensor_scan(
                    out=av[:, :], data0=qt[:, :], data1=av[:, :], initial=0.0,
                    op0=Alu.mult, op1=Alu.add,
                )
                nc.vector.tensor_tensor_scan(
                    out=avr[:, :], data0=qt[:, :], data1=avr[:, :], initial=0.0,
                    op0=Alu.mult, op1=Alu.add,
                )

                # ---- combine: out[c, t, 30+f] = yf[c, f, t] + yb[c, f, T-1-t]
                nc.vector.tensor_tensor(
                    out=mk(outt, F, [[T * OC, CPT], [OC, T], [1, F]]),
                    in0=mk(av, 0, [[FT, CPT], [1, T], [T, F]]),
                    in1=mk(avr, T - 1, [[FT, CPT], [-1, T], [T, F]]),
                    op=Alu.add,
                )

                # ---- spatial passthrough: out[c, t, f] = x[c, t, f] (GpSimd) ----
                nc.gpsimd.tensor_copy(
                    out=mk(outt, 0, [[T * OC, CPT], [OC, T], [1, F]]),
                    in_=mk(xin, 0, [[FT, CPT], [F, T], [1, F]]),
                )

                # ---- M-Zeni channel ----
                # forward dir from pelvis displacement (joint 0, feats 0..2):
                # fdir = pd / (||pd|| + 63e-6), pd = p[T-1] - p[0]
                pd = psmall.tile([128, CPT * 3], f32)
                sq = psmall.tile([128, CPT * 3], f32)
                nsq = psmall.tile([128, CPT], f32)
                inv = psmall.tile([128, CPT], f32)
                fd = psmall.tile([128, CPT * 3], f32)
                ad = psmall.tile([128, CPT * T * 3], f32)
                mzt = psmall.tile([128, CPT * T], f32)

                pd3 = mk(pd, 0, [[3, CPT], [1, 3]])
                nc.vector.tensor_tensor(
                    out=pd3,
                    in0=mk(xin, (T - 1) * F, [[FT, CPT], [1, 3]]),
                    in1=mk(xin, 0, [[FT, CPT], [1, 3]]),
                    op=Alu.subtract,
                )
                nc.vector.tensor_tensor(out=mk(sq, 0, [[3, CPT], [1, 3]]),
                                         in0=pd3, in1=pd3, op=Alu.mult)
                nc.vector.tensor_reduce(
                    out=mk(nsq, 0, [[1, CPT]]),
                    in_=mk(sq, 0, [[3, CPT], [1, 3]]),
                    axis=mybir.AxisListType.X, op=Alu.add,
                )
                ns2 = mk(nsq, 0, [[1, CPT]])
                nc.scalar.sqrt(out=ns2, in_=ns2)
                nc.vector.tensor_scalar_add(out=ns2, in0=ns2, scalar1=(T - 1) * 1e-6)
                nc.vector.reciprocal(mk(inv, 0, [[1, CPT]]), ns2)
                nc.vector.tensor_tensor(
                    out=mk(fd, 0, [[3, CPT], [1, 3]]), in0=pd3,
                    in1=mk(inv, 0, [[1, CPT], [0, 3]]), op=Alu.mult,
                )
                # ankle_l (joint 3, feats 9..11) - ankle_r (joint 6, feats 18..20)
                ad3 = mk(ad, 0, [[T * 3, CPT], [3, T], [1, 3]])
                nc.vector.tensor_tensor(
                    out=ad3,
                    in0=mk(xin, 9, [[FT, CPT], [F, T], [1, 3]]),
                    in1=mk(xin, 18, [[FT, CPT], [F, T], [1, 3]]),
                    op=Alu.subtract,
                )
                nc.vector.tensor_tensor(
                    out=ad3, in0=ad3,
                    in1=mk(fd, 0, [[3, CPT], [0, T], [1, 3]]), op=Alu.mult,
                )
                nc.vector.tensor_reduce(
                    out=mk(mzt, 0, [[T, CPT], [1, T]]),
                    in_=ad3, axis=mybir.AxisListType.X, op=Alu.add,
                )
                nc.vector.tensor_scalar_add(
                    out=mk(outt, 2 * F, [[T * OC, CPT], [OC, T]]),
                    in0=mk(mzt, 0, [[T, CPT], [1, T]]), scalar1=1e-6,
                )

                # ---- store (SP-issued HWDGE ring) ----
                out_sl = outd[r0 : r0 + ROWS, :].rearrange(
                    "(c p) f -> p c f", p=128
                )
                nc.sync.dma_start(
                    out=out_sl, in_=mk(outt, 0, [[T * OC, CPT], [1, T * OC]])
                )

    nc.compile()
    return nc


_CACHE = {}


def _get_nc():
    if "nc" not in _CACHE:
        _CACHE["nc"] = build_nc(BC)
    return _CACHE["nc"]


def kernel(pose_seq: np.ndarray, dt_seq: np.ndarray) -> np.ndarray:
    from concourse.bass_utils import run_bass_kernel_spmd

    nc = _get_nc()
    pose = np.ascontiguousarray(
        pose_seq.reshape(B, FT), dtype=np.float32
    )
    dt = np.ascontiguousarray(dt_seq.reshape(B, T), dtype=np.float32)

    in_maps = [
        {
            "pose": pose[c * BC : (c + 1) * BC],
            "dt": dt[c * BC : (c + 1) * BC],
        }
        for c in range(N_CORES)
    ]
    res = run_bass_kernel_spmd(nc, in_maps, list(range(N_CORES)))
    out = np.concatenate([r["out"] for r in res.results], axis=0)
    return out.reshape(B, T, OC)


# revision 7
# speedup vs baseline: 7.4136x; 7.4136x over previous
"""Trainium2 Bass kernel for nn_PhysicsPriorExtractor.

Reference computation per batch element (B=32768, T=64, K*D=30 features):
  spatial = pose reshaped [T, 30]                          -> out channels 0..29
  vel     = bidirectional-EMA(diff(spatial)/clip(dt))      -> out channels 30..59
  mzeni   = dot(ankle_l - ankle_r, forward_dir) + 1e-6     -> out channel 60

Strategy (pure data-parallel, batch sharded 8 ways):
  * batch on SBUF partitions, CPT=2 batch rows per partition per tile
  * EMA scans via the DVE TensorTensorScan instruction, all 30 features
    (x CPT batches) in ONE scan: feature-major [F, T] free layout with the
    recurrence reset at segment starts via data0=0 there (state = d0*state + d1)
  * backward scan runs on a time-reversed copy (ACT engine negative-stride
    copy); the combine pass reads it back reversed
  * everything assembled into a [128, CPT*64*61] output tile, one big
    contiguous DMA per tile -> memory-bound roofline
"""

import sys

if "/opt/trn_rl_repo" not in sys.path:
    sys.path.insert(0, "/opt/trn_rl_repo")

import numpy as np

B, T, F = 32768, 64, 30
OC = 2 * F + 1  # 61 output channels
N_CORES = 8
BC = B // N_CORES  # 4096 batch rows per core
CPT = 2  # batch rows per partition per tile
ROWS = 128 * CPT  # batch rows per tile
ALPHA = 0.7
Q = 1.0 - ALPHA
MAX_DT = 0.1

FT = F * T  # 1920, per-batch free width (scan layout)
W = CPT * FT  # 3840
OW = CPT * T * OC  # 7808


def build_nc(bc=BC, repeat=1, loop_repeat=1):
    """Build the single-core Bass program processing a [bc, T, F] shard.

    repeat>1 unrolls the whole body that many times; loop_repeat>1 wraps it
    in a hardware For_i loop (timing-only variants; extra passes recompute
    the same output)."""
    import concourse.bacc as bacc
    import concourse.mybir as mybir
    from concourse import tile
    from concourse.ap import AP

    f32 = mybir.dt.float32
    Alu = mybir.AluOpType
    n_tiles = bc // ROWS
    assert n_tiles * ROWS == bc

    nc = bacc.Bacc(
        "TRN2", target_bir_lowering=False, debug=False, num_devices=N_CORES
    )
    pose = nc.dram_tensor("pose", [bc, FT], f32, kind="ExternalInput")
    dts = nc.dram_tensor("dt", [bc, T], f32, kind="ExternalInput")
    outd = nc.dram_tensor("out", [bc, T * OC], f32, kind="ExternalOutput")

    def mk(t, off, pairs):
        # custom free-dim access pattern on a 2D [128, N] tile
        return AP(t.tensor, t.offset + off, [list(t.ap[0])] + [list(p) for p in pairs])

    with tile.TileContext(nc) as tc:
        with (
            tc.tile_pool(name="cpool", bufs=1) as cpool,
            tc.tile_pool(name="pin", bufs=3) as pin,
            tc.tile_pool(name="pmid", bufs=2) as pmid,
            tc.tile_pool(name="pout", bufs=2) as pout,
            tc.tile_pool(name="psmall", bufs=2) as psmall,
        ):
            # scan decay tile: Q everywhere, 0 at each segment start so the
            # recurrence resets per (batch-chunk, feature) segment
            qt = cpool.tile([128, W], f32)
            nc.vector.memset(qt[:, :], Q)
            nc.vector.memset(mk(qt, 0, [[FT, CPT], [T, F]]), 0.0)

            def body():
                for i in range(n_tiles * repeat):
                    _tile_body(i)

            def _tile_body(i):
                r0 = (i % n_tiles) * ROWS
                xin = pin.tile([128, W], f32)
                dtt = psmall.tile([128, CPT * T], f32)
                rr = psmall.tile([128, CPT * T], f32)
                av = pmid.tile([128, W], f32)
                avr = pmid.tile([128, W], f32)
                outt = pout.tile([128, OW], f32)

                # ---- loads (ACT-issued HWDGE ring, separate from stores) ----
                pose_sl = pose[r0 : r0 + ROWS, :].rearrange(
                    "(c p) f -> p c f", p=128
                )
                nc.scalar.dma_start(out=mk(xin, 0, [[FT, CPT], [1, FT]]), in_=pose_sl)
                dt_sl = dts[r0 : r0 + ROWS, :].rearrange("(c p) t -> p c t", p=128)
                nc.scalar.dma_start(out=mk(dtt, 0, [[T, CPT], [1, T]]), in_=dt_sl)

                # ---- rr = 1 / clip(dt, 1e-6, MAX_DT) ----
                nc.vector.tensor_scalar(
                    out=rr[:, :], in0=dtt[:, :], scalar1=MAX_DT, scalar2=1e-6,
                    op0=Alu.min, op1=Alu.max,
                )
                nc.vector.reciprocal(rr[:, :], rr[:, :])

                # ---- av[c, f, t] = (x[c, t, f] - x[c, t-1, f]) * (0.5*alpha/dt_t)
                # x is t-major in xin; av is written f-major (T contiguous)
                av_b = mk(av, 1, [[FT, CPT], [T, F], [1, T - 1]])
                nc.vector.tensor_tensor(
                    out=av_b,
                    in0=mk(xin, F, [[FT, CPT], [1, F], [F, T - 1]]),
                    in1=mk(xin, 0, [[FT, CPT], [1, F], [F, T - 1]]),
                    op=Alu.subtract,
                )
                # TensorScalarPtr ops are limited to 3D APs by the BIR
                # verifier -> one scale op per batch chunk
                for c in range(CPT):
                    avc = mk(av, c * FT + 1, [[T, F], [1, T - 1]])
                    nc.vector.scalar_tensor_tensor(
                        out=avc, in0=avc, scalar=0.5 * ALPHA,
                        in1=mk(rr, c * T + 1, [[0, F], [1, T - 1]]),
                        op0=Alu.mult, op1=Alu.mult,
                    )
                # vel_0 = 0
                nc.vector.memset(mk(av, 0, [[FT, CPT], [T, F]]), 0.0)

                # ---- time-reversed copy for the backward scan (ACT) ----
                nc.scalar.copy(
                    out=mk(avr, 0, [[FT, CPT], [T, F], [1, T]]),
                    in_=mk(av, T - 1, [[FT, CPT], [T, F], [-1, T]]),
                )
                # backward initial condition: z_{T-1} = v_{T-1} (not alpha*v):
                # patch segment starts from 0.5*alpha*v to 0.5*v
                seg0 = mk(avr, 0, [[FT, CPT], [T, F]])
                nc.vector.tensor_scalar_mul(out=seg0, in0=seg0, scalar1=1.0 / ALPHA)

                # ---- the two EMA scans (in-place), state = q*state + d1 ----
                nc.vector.tensor_tensor_scan(
                    out=av[:, :], data0=qt[:, :], data1=av[:, :], initial=0.0,
                    op0=Alu.mult, op1=Alu.add,
                )
                nc.vector.tensor_tensor_scan(
                    out=avr[:, :], data0=qt[:, :], data1=avr[:, :], initial=0.0,
                    op0=Alu.mult, op1=Alu.add,
                )

                # ---- combine: out[c, t, 30+f] = yf[c, f, t] + yb[c, f, T-1-t]
                nc.vector.tensor_tensor(
                    out=mk(outt, F, [[T * OC, CPT], [OC, T], [1, F]]),
                    in0=mk(av, 0, [[FT, CPT], [1, T], [T, F]]),
                    in1=mk(avr, T - 1, [[FT, CPT], [-1, T], [T, F]]),
                    op=Alu.add,
                )

                # ---- spatial passthrough: out[c, t, f] = x[c, t, f] (GpSimd) ----
                nc.gpsimd.tensor_copy(
                    out=mk(outt, 0, [[T * OC, CPT], [OC, T], [1, F]]),
                    in_=mk(xin, 0, [[FT, CPT], [F, T], [1, F]]),
                )

                # ---- M-Zeni channel ----
                # forward dir from pelvis displacement (joint 0, feats 0..2):
                # fdir = pd / (||pd|| + 63e-6), pd = p[T-1] - p[0]
                pd = psmall.tile([128, CPT * 3], f32)
                sq = psmall.tile([128, CPT * 3], f32)
                nsq = psmall.tile([128, CPT], f32)
                inv = psmall.tile([128, CPT], f32)
                fd = psmall.tile([128, CPT * 3], f32)
                ad = psmall.tile([128, CPT * T * 3], f32)
                mzt = psmall.tile([128, CPT * T], f32)

                pd3 = mk(pd, 0, [[3, CPT], [1, 3]])
                nc.vector.tensor_tensor(
                    out=pd3,
                    in0=mk(xin, (T - 1) * F, [[FT, CPT], [1, 3]]),
                    in1=mk(xin, 0, [[FT, CPT], [1, 3]]),
                    op=Alu.subtract,
                )
                nc.vector.tensor_tensor(out=mk(sq, 0, [[3, CPT], [1, 3]]),
                                         in0=pd3, in1=pd3, op=Alu.mult)
                nc.vector.tensor_reduce(
                    out=mk(nsq, 0, [[1, CPT]]),
                    in_=mk(sq, 0, [[3, CPT], [1, 3]]),
                    axis=mybir.AxisListType.X, op=Alu.add,
                )
                ns2 = mk(nsq, 0, [[1, CPT]])
                nc.scalar.sqrt(out=ns2, in_=ns2)
                nc.vector.tensor_scalar_add(out=ns2, in0=ns2, scalar1=(T - 1) * 1e-6)
                nc.vector.reciprocal(mk(inv, 0, [[1, CPT]]), ns2)
                nc.vector.tensor_tensor(
                    out=mk(fd, 0, [[3, CPT], [1, 3]]), in0=pd3,
                    in1=mk(inv, 0, [[1, CPT], [0, 3]]), op=Alu.mult,
                )
                # ankle_l (joint 3, feats 9..11) - ankle_r (joint 6, feats 18..20)
                ad3 = mk(ad, 0, [[T * 3, CPT], [3, T], [1, 3]])
                nc.vector.tensor_tensor(
                    out=ad3,
                    in0=mk(xin, 9, [[FT, CPT], [F, T], [1, 3]]),
                    in1=mk(xin, 18, [[FT, CPT], [F, T], [1, 3]]),
                    op=Alu.subtract,
                )
                nc.vector.tensor_tensor(
                    out=ad3, in0=ad3,
                    in1=mk(fd, 0, [[3, CPT], [0, T], [1, 3]]), op=Alu.mult,
                )
                nc.vector.tensor_reduce(
                    out=mk(mzt, 0, [[T, CPT], [1, T]]),
                    in_=ad3, axis=mybir.AxisListType.X, op=Alu.add,
                )
                nc.vector.tensor_scalar_add(
                    out=mk(outt, 2 * F, [[T * OC, CPT], [OC, T]]),
                    in0=mk(mzt, 0, [[T, CPT], [1, T]]), scalar1=1e-6,
                )

                # ---- store (SP-issued HWDGE ring) ----
                out_sl = outd[r0 : r0 + ROWS, :].rearrange(
                    "(c p) f -> p c f", p=128
                )
                nc.sync.dma_start(
                    out=out_sl, in_=mk(outt, 0, [[T * OC, CPT], [1, T * OC]])
                )

            if loop_repeat > 1:
                with tc.For_i(0, loop_repeat, 1):
                    body()
            else:
                body()

    nc.compile()
    return nc


_CACHE = {}


def _get_nc():
    if "nc" not in _CACHE:
        _CACHE["nc"] = build_nc(BC)
    return _CACHE["nc"]


def kernel(pose_seq: np.ndarray, dt_seq: np.ndarray) -> np.ndarray:
    from concourse.bass_utils import run_bass_kernel_spmd

    nc = _get_nc()
    pose = np.ascontiguousarray(
        pose_seq.reshape(B, FT), dtype=np.float32
    )
    dt = np.ascontiguousarray(dt_seq.reshape(B, T), dtype=np.float32)

    in_maps = [
        {
            "pose": pose[c * BC : (c + 1) * BC],
            "dt": dt[c * BC : (c + 1) * BC],
        }
        for c in range(N_CORES)
    ]
    res = run_bass_kernel_spmd(nc, in_maps, list(range(N_CORES)))
    out = np.concatenate([r["out"] for r in res.results], axis=0)
    return out.reshape(B, T, OC)
